# revision 1
# baseline (speedup 1.0000x reference)
"""Pairwise squared-distance kernel for Trainium2 (8 NeuronCores).

out[i, j] = mean_d (x_i[d] - y_j[d])^2
          = (||x_i||^2 + ||y_j||^2 - 2 x_i . y_j) / D

Sharding: rows of z_queries split across 8 cores (1024 rows each);
class_prototypes replicated. Each core computes its [1024, 4096] slab.

Device kernel (per core):
  - inputs pre-transposed on host to [D, rows] so the contraction dim is
    the SBUF partition dim (clean contiguous DMAs, no on-chip transpose).
  - prototypes pre-scaled by -2/D (= -2^-8, exact) so PSUM accumulates
    -2/D * x.y directly.
  - GEMM: for each (m-tile 128 queries, n-half 2048 protos): 4 k-tiles x
    4 n-subtiles of matmul into a [128, 2048] PSUM tile (4 banks).
  - epilogue: one DVE scalar_tensor_tensor: (psum + xsq/D[i]) + ysq/D[j].
  - 1 MiB output DMAs.
"""

import sys

if "/opt/trn_rl_repo" not in sys.path:
    sys.path.insert(0, "/opt/trn_rl_repo")

import numpy as np

N_CORES = 8
N_Q = 8192
N_P = 4096
D = 512
ROWS = N_Q // N_CORES  # 1024 query rows per core

P = 128
M_TILES = ROWS // P  # 8
K_TILES = D // P  # 4
N_BLOCK = 1024  # psum group free dim (2 banks of fp32)
N_BLOCKS = N_P // N_BLOCK  # 4
NB = 512  # matmul free dim (1 psum bank)
NSUB = N_BLOCK // NB  # 2
WAVE = 4  # m-tiles per wave (4 x 2 banks = 8 PSUM banks)
N_WAVES = M_TILES // WAVE  # 2

# "bf16" halves input DMA traffic; "f32r" keeps fp32 inputs at TF32 matmul rate.
COMPUTE_DT = "bf16"

_CACHE = {}


def _build_nc(compute_dt: str):
    import concourse.mybir as mybir
    import concourse.tile as tile
    from concourse import bacc

    if compute_dt == "bf16":
        in_dt = mybir.dt.bfloat16
        mm_cast = lambda ap: ap
    elif compute_dt == "f32r":
        in_dt = mybir.dt.float32
        mm_cast = lambda ap: ap.bitcast(mybir.dt.float32r)
    else:
        raise ValueError(compute_dt)

    f32 = mybir.dt.float32
    add = mybir.AluOpType.add

    nc = bacc.Bacc("TRN2", target_bir_lowering=False, debug=False, num_devices=N_CORES)

    # qp packs [qt | pt] along the free dim so one DMA chunk [qt_k | pt_nb0_k]
    # unlocks the first k-sweep with a single completion.
    qp = nc.dram_tensor("qp", (D, ROWS + N_P), in_dt, kind="ExternalInput")
    ab = nc.dram_tensor("ab", (P, M_TILES), f32, kind="ExternalInput")
    bb = nc.dram_tensor("bb", (1, N_P), f32, kind="ExternalInput")
    out = nc.dram_tensor("out", (ROWS, N_P), f32, kind="ExternalOutput")
    N_FRONT = ROWS + NB  # 1536: qt_k | pt_block0_k
    N_REST = N_P - 2 * NB  # 3072: pt blocks 2..7
    NBLK = N_P // NB  # 8 column blocks of 512

    with tile.TileContext(nc) as tc:
        with (
            tc.tile_pool(name="inputs", bufs=1) as in_pool,
            tc.tile_pool(name="outs", bufs=8) as out_pool,
            tc.tile_pool(name="psum", bufs=8, space="PSUM") as psum_pool,
        ):
            # All inputs ride the sync ring (q1) in exact consumption order —
            # the two HWDGE rings don't round-robin fairly (q1 starves q10),
            # so FIFO position on q1 IS the data priority. Outputs go to the
            # scalar ring (q10), which naturally yields to input traffic.
            qt_tiles = [None] * K_TILES
            ptb = [[None] * K_TILES for _ in range(NBLK)]

            def load_front(k):
                fr_t = in_pool.tile([P, N_FRONT], in_dt, name=f"front_{k}")
                nc.sync.dma_start(out=fr_t, in_=qp[k * P : (k + 1) * P, 0:N_FRONT])
                qt_tiles[k] = fr_t[:, 0:ROWS]
                ptb[0][k] = fr_t[:, ROWS:N_FRONT]

            def load_b1(k):
                b1_t = in_pool.tile([P, NB], in_dt, name=f"b1_{k}")
                nc.sync.dma_start(
                    out=b1_t, in_=qp[k * P : (k + 1) * P, N_FRONT : N_FRONT + NB]
                )
                ptb[1][k] = b1_t

            def load_rest(k):
                re_t = in_pool.tile([P, N_REST], in_dt, name=f"rest_{k}")
                nc.sync.dma_start(
                    out=re_t, in_=qp[k * P : (k + 1) * P, N_FRONT + NB : ROWS + N_P]
                )
                for b in range(2, NBLK):
                    ptb[b][k] = re_t[:, (b - 2) * NB : (b - 1) * NB]

            load_front(0)
            # b row early (tiny); its on-device partition broadcast (saves a
            # 2 MiB HBM load) runs on GpSimd during the input stream.
            brow_t = in_pool.tile([1, N_P], f32, name="brow_t")
            nc.sync.dma_start(out=brow_t, in_=bb[0:1, :])
            bb_t = in_pool.tile([P, N_P], f32, name="bb_t")
            nc.gpsimd.partition_broadcast(bb_t, brow_t)
            for k in range(1, K_TILES):
                load_front(k)
            for k in range(K_TILES):
                load_b1(k)
            ab_t = in_pool.tile([P, M_TILES], f32, name="ab_t")
            nc.sync.dma_start(out=ab_t, in_=ab[:, :])
            for k in range(K_TILES):
                load_rest(k)

            n_out = 0

            def epilogue(psum_t, m, b):
                nonlocal n_out
                out_t = out_pool.tile([P, NB], f32, name="out_t")
                # out = (psum + xsq/D[i]) + ysq/D[j]
                nc.vector.scalar_tensor_tensor(
                    out=out_t,
                    in0=psum_t,
                    scalar=ab_t[:, m : m + 1],
                    in1=bb_t[:, b * NB : (b + 1) * NB],
                    op0=add,
                    op1=add,
                )
                out_eng = nc.scalar if n_out % 2 == 0 else nc.sync
                n_out += 1
                out_eng.dma_start(
                    out=out[m * P : (m + 1) * P, b * NB : (b + 1) * NB],
                    in_=out_t,
                )

            def mm(psum_t, m, b, k):
                nc.tensor.matmul(
                    psum_t,
                    mm_cast(qt_tiles[k][:, m * P : (m + 1) * P]),
                    mm_cast(ptb[b][k]),
                    start=(k == 0),
                    stop=(k == K_TILES - 1),
                )

            # Block 0: k-outer / m-inner over all 8 m-tiles (8 one-bank PSUM
            # groups) — each newly-landed chunk unlocks a full 8-matmul
            # sweep, so the PE goes dense while inputs are still streaming.
            psums = [
                psum_pool.tile([P, NB], f32, name="ps", tag="ps")
                for _ in range(M_TILES)
            ]
            for k in range(K_TILES):
                for m in range(M_TILES):
                    mm(psums[m], m, 0, k)
            for m in range(M_TILES):
                epilogue(psums[m], m, 0)

            # Blocks 1-7: everything is resident by then — m-outer / k-inner,
            # so each group's epilogue pipelines under the next group's
            # matmuls (no 8-deep epilogue pile-up blocking PSUM recycling),
            # and the kernel tail is a single small epilogue + 256 KiB store.
            for b in range(1, NBLK):
                for m in range(M_TILES):
                    psum_t = psum_pool.tile([P, NB], f32, name="ps", tag="ps")
                    for k in range(K_TILES):
                        mm(psum_t, m, b, k)
                    epilogue(psum_t, m, b)

    nc.compile()
    return nc


def _get_nc(compute_dt: str):
    if compute_dt not in _CACHE:
        _CACHE[compute_dt] = _build_nc(compute_dt)
    return _CACHE[compute_dt]


def _prep_inputs(z_queries: np.ndarray, class_prototypes: np.ndarray, compute_dt: str):
    import ml_dtypes

    np_in = ml_dtypes.bfloat16 if compute_dt == "bf16" else np.float32

    z = np.ascontiguousarray(z_queries, dtype=np.float32)
    p = np.ascontiguousarray(class_prototypes, dtype=np.float32)

    a = (z.astype(np.float64) ** 2).sum(axis=1) / D  # (N_Q,) ||x||^2 / D
    b = (p.astype(np.float64) ** 2).sum(axis=1) / D  # (N_P,) ||y||^2 / D

    pt = (p.T * np.float32(-2.0 / D)).astype(np_in)  # [D, N_P]
    bb = np.ascontiguousarray(b.astype(np.float32).reshape(1, N_P))  # [1, N_P]

    in_maps = []
    for c in range(N_CORES):
        sl = slice(c * ROWS, (c + 1) * ROWS)
        qt_c = z[sl].T.astype(np_in)  # [D, ROWS]
        qp_c = np.ascontiguousarray(np.concatenate([qt_c, pt], axis=1))  # [D, ROWS+N_P]
        ab_c = np.ascontiguousarray(
            a[sl].astype(np.float32).reshape(M_TILES, P).T
        )  # [P, M_TILES]
        in_maps.append({"qp": qp_c, "ab": ab_c, "bb": bb})
    return in_maps


def run(z_queries, class_prototypes, compute_dt=COMPUTE_DT, **spmd_kwargs):
    from concourse.bass_utils import run_bass_kernel_spmd

    nc = _get_nc(compute_dt)
    in_maps = _prep_inputs(z_queries, class_prototypes, compute_dt)
    res = run_bass_kernel_spmd(nc, in_maps, core_ids=list(range(N_CORES)), **spmd_kwargs)
    full = np.concatenate([r["out"] for r in res.results], axis=0)
    return full, res


def kernel(z_queries: np.ndarray, class_prototypes: np.ndarray) -> np.ndarray:
    full, _ = run(z_queries, class_prototypes)
    return full



# revision 2
# speedup vs baseline: 1.4287x; 1.4287x over previous
"""Pairwise squared-distance kernel for Trainium2 (8 NeuronCores).

out[i, j] = mean_d (x_i[d] - y_j[d])^2
          = (||x_i||^2 + ||y_j||^2 - 2 x_i . y_j) / D

Sharding: rows of z_queries split across 8 cores (1024 rows each);
class_prototypes replicated. Each core computes its [1024, 4096] slab.

v2 design (fp8 DoubleRow):
  - device computes ONLY the cross-term GEMM x.y in fp8e4m3 with
    perf_mode=DoubleRow (contraction 256 per matmul -> 128 MMs/core
    instead of 256, ~1.4x PE throughput vs bf16).
  - norm terms ||x||^2/D, ||y||^2/D are rank-1 updates: computed on
    host in fp64 and added during unshard (device epilogue is just a
    scaled cast psum * (-2/D) -> bf16).
  - inputs pre-packed on host to [128, k_sub, cols] fp8 (2.6 MB/core),
    output stored bf16 (8.4 MB/core) -> total HBM 11 MB/core vs 22.
  - epilogue alternates ScalarE (activation copy w/ scale) and VectorE
    (tensor_scalar_mul); output DMAs alternate the q10/q1 rings.
"""

import sys

if "/opt/trn_rl_repo" not in sys.path:
    sys.path.insert(0, "/opt/trn_rl_repo")

import numpy as np

N_CORES = 8
N_Q = 8192
N_P = 4096
D = 512
ROWS = N_Q // N_CORES  # 1024 query rows per core
P = 128
M_TILES = ROWS // P  # 8
K_SUB = D // P  # 4 k-subtiles of 128
NB = 512  # matmul free dim (out), 1 psum bank
NBLK = N_P // NB  # 8 proto column blocks
SCALE = -2.0 / D  # folded into the epilogue (exact power of two)

_CACHE = {}


def _build_nc():
    import concourse.mybir as mybir
    import concourse.tile as tile
    from concourse import bacc

    fp8 = mybir.dt.float8e4
    bf16 = mybir.dt.bfloat16
    f32 = mybir.dt.float32
    DR = mybir.MatmulPerfMode.DoubleRow

    nc = bacc.Bacc("TRN2", target_bir_lowering=False, debug=False, num_devices=N_CORES)

    # q8[p, m*4+k, r] = x8[m*128+r, k*128+p]; p8[p, b*4+k, n] = y8[b*512+n, k*128+p]
    q8 = nc.dram_tensor("q8", (P, M_TILES * K_SUB, P), fp8, kind="ExternalInput")
    p8 = nc.dram_tensor("p8", (P, NBLK * K_SUB, NB), fp8, kind="ExternalInput")
    out = nc.dram_tensor("out", (ROWS, N_P), bf16, kind="ExternalOutput")

    with tile.TileContext(nc) as tc:
        with (
            tc.tile_pool(name="inputs", bufs=1) as in_pool,
            tc.tile_pool(name="outs", bufs=12) as out_pool,
            tc.tile_pool(name="psum", bufs=8, space="PSUM") as psum_pool,
        ):
            # Inputs ride the sync ring (q1) in exact consumption order.
            # First MM needs qt_lo + ptb[0] (~512 KB); q_hi needed ~2 us in.
            qt_lo = in_pool.tile([P, 4 * K_SUB, P], fp8, name="qt_lo")
            nc.sync.dma_start(out=qt_lo, in_=q8[:, 0 : 4 * K_SUB, :])
            ptb = []
            pt0 = in_pool.tile([P, K_SUB, NB], fp8, name="pt0")
            nc.sync.dma_start(out=pt0, in_=p8[:, 0:K_SUB, :])
            ptb.append(pt0)
            qt_hi = in_pool.tile([P, 4 * K_SUB, P], fp8, name="qt_hi")
            nc.sync.dma_start(out=qt_hi, in_=q8[:, 4 * K_SUB : 8 * K_SUB, :])
            for b in range(1, NBLK):
                pb_t = in_pool.tile([P, K_SUB, NB], fp8, name=f"pt{b}")
                nc.sync.dma_start(out=pb_t, in_=p8[:, b * K_SUB : (b + 1) * K_SUB, :])
                ptb.append(pb_t)

            def qt_slice(m, g):
                t = qt_lo if m < 4 else qt_hi
                i = (m % 4) * K_SUB + 2 * g
                return t[:, i : i + 2, :]

            n_out = 0
            for b in range(NBLK):
                for m in range(M_TILES):
                    psum_t = psum_pool.tile([P, NB], f32, name="ps", tag="ps")
                    for g in range(2):
                        nc.tensor.matmul(
                            psum_t,
                            qt_slice(m, g),
                            ptb[b][:, 2 * g : 2 * g + 2, :],
                            start=(g == 0),
                            stop=(g == 1),
                            perf_mode=DR,
                        )
                    out_t = out_pool.tile([P, NB], bf16, name="out_t")
                    if n_out % 2 == 0:
                        nc.scalar.mul(out_t, psum_t, SCALE)
                        out_eng = nc.scalar
                    else:
                        nc.vector.tensor_scalar_mul(out_t, psum_t, SCALE)
                        out_eng = nc.sync
                    n_out += 1
                    out_eng.dma_start(
                        out=out[m * P : (m + 1) * P, b * NB : (b + 1) * NB],
                        in_=out_t,
                    )

    nc.compile()
    return nc


def _get_nc():
    if "nc" not in _CACHE:
        _CACHE["nc"] = _build_nc()
    return _CACHE["nc"]


def _prep_inputs(z_queries: np.ndarray, class_prototypes: np.ndarray):
    import ml_dtypes

    fp8 = ml_dtypes.float8_e4m3

    z = np.ascontiguousarray(z_queries, dtype=np.float32)
    p = np.ascontiguousarray(class_prototypes, dtype=np.float32)

    a = (z.astype(np.float64) ** 2).sum(axis=1) / D  # (N_Q,) ||x||^2 / D
    b = (p.astype(np.float64) ** 2).sum(axis=1) / D  # (N_P,) ||y||^2 / D

    y8 = p.astype(fp8)  # [N_P, D]
    # p8[p, b*4+k, n] = y8[b*512+n, k*128+p]
    p8 = np.ascontiguousarray(
        y8.reshape(NBLK, NB, K_SUB, P).transpose(3, 0, 2, 1).reshape(P, NBLK * K_SUB, NB)
    )

    in_maps = []
    for c in range(N_CORES):
        sl = slice(c * ROWS, (c + 1) * ROWS)
        x8 = z[sl].astype(fp8)  # [ROWS, D]
        # q8[p, m*4+k, r] = x8[m*128+r, k*128+p]
        q8_c = np.ascontiguousarray(
            x8.reshape(M_TILES, P, K_SUB, P)
            .transpose(3, 0, 2, 1)
            .reshape(P, M_TILES * K_SUB, P)
        )
        in_maps.append({"q8": q8_c, "p8": p8})
    return in_maps, a.astype(np.float32), b.astype(np.float32)


def run(z_queries, class_prototypes, **spmd_kwargs):
    from concourse.bass_utils import run_bass_kernel_spmd

    nc = _get_nc()
    in_maps, a, b = _prep_inputs(z_queries, class_prototypes)
    res = run_bass_kernel_spmd(nc, in_maps, core_ids=list(range(N_CORES)), **spmd_kwargs)
    full = np.concatenate(
        [np.asarray(r["out"]) for r in res.results], axis=0
    ).astype(np.float32)
    full += a[:, None]
    full += b[None, :]
    return full, res


def kernel(z_queries: np.ndarray, class_prototypes: np.ndarray) -> np.ndarray:
    full, _ = run(z_queries, class_prototypes)
    return full
